# revision 1
# baseline (speedup 1.0000x reference)
"""Trainium2 Bass kernel for nn_BlurLayer (B=128, 224x224x3, per-sample
rotated-line motion blur, SAME depthwise conv).

Self-contained: kernel(**inputs) -> np.ndarray. Shards the batch over 8
NeuronCores (pure data parallel: 16 samples per core), compiles + runs one
SPMD Bass program via concourse.bass_utils.run_bass_kernel_spmd, gathers
the full output.

Method: the rotated blur kernel's nonzero taps all equal 1/size and form a
digitized line. Per sample we pick the basis (identity / transpose /
unit shear, applied to the kernel) that minimizes PE streaming cost: taps
grouped by (sheared) kernel column give banded 0/1 weight matrices
contracted over image rows on the PE. The image is split into two fp8e4m3
planes (hi = fp8(x), lo = fp8(x - hi)) so each matmul runs in DoubleRow
perf mode (2 fp8 MACs/PE/cycle). Horizontal alignment and the shear are
baked into the host-side blob layout (a shear is linear in the row index),
so all device access patterns are static; sheared outputs are written to
DRAM in sheared form and unsheared on the host. Each slot's image windows
+ weight table arrive in one DMA; a post-compile pass drops back-to-back
reloads of identical PE weights; 1/size scaling happens in the PSUM->SBUF
copies, split between the Scalar and Vector engines.
"""

import math

import numpy as np
import ml_dtypes

MAXK = 32
H = W = 224
C = 3
WC = W * C                  # 672
PAD_LO = (MAXK - 1) // 2    # 15
PIMG_PAD = 800              # left zero margin (elems) of padded image row
PIMG_W = PIMG_PAD + WC + 800
WEDGE = 3 * 111             # extra sheared-output cols per 112-row block

FP8 = ml_dtypes.float8_e4m3


def dedupe_ldweights(nc):
    """Replace an InstLdweights whose weights AP is identical to the
    immediately-preceding one (per block) with an InstNoOp carrying its
    sync_info: the PE array still holds those weights, so the reload is
    pure Tensor-queue overhead (~158ns each)."""
    import concourse.mybir as mybir
    n = 0
    for fn in nc.m.functions:
        for blk in fn.blocks:
            prev_key = None
            new_insts = []
            for inst in blk.instructions:
                if isinstance(inst, mybir.InstLdweights):
                    key = (repr(inst.ins), repr(getattr(inst, "perf_mode", None)),
                           repr(getattr(inst, "is_transpose", None)))
                    if key == prev_key:
                        n += 1
                        inst = mybir.InstNoOp(
                            name=f"{inst.name}-ldwdedup",
                            engine=inst.engine,
                            ins=[], outs=[],
                            sync_info=inst.sync_info,
                        )
                    else:
                        prev_key = key
                new_insts.append(inst)
            blk.instructions = new_insts
    return n


# ---------------------------------------------------------------- host math
def rotate_nearest_np(img, rad):
    K = img.shape[0]
    cos, sin = np.cos(rad), np.sin(rad)
    coords = np.arange(K, dtype=np.float32)
    yy, xx = np.meshgrid(coords, coords, indexing="ij")
    e = np.float32(K - 1)
    x_off = (e - (cos * e - sin * e)) * 0.5
    y_off = (e - (sin * e + cos * e)) * 0.5
    sx = cos * xx - sin * yy + x_off
    sy = sin * xx + cos * yy + y_off
    ix = np.round(sx).astype(np.int32)
    iy = np.round(sy).astype(np.int32)
    valid = (ix >= 0) & (ix < K) & (iy >= 0) & (iy < K)
    g = img[np.clip(iy, 0, K - 1), np.clip(ix, 0, K - 1)]
    return np.where(valid, g, np.float32(0.0))


def _col_groups(ker):
    """Group nonzero taps of `ker` by column -> [(kx, klo, khi)], splitting
    any non-contiguous run."""
    ys, xs = np.nonzero(ker)
    groups = []
    for kx in np.unique(xs):
        run = np.sort(ys[xs == kx])
        start = prev = int(run[0])
        for v in run[1:]:
            v = int(v)
            if v == prev + 1:
                prev = v
            else:
                groups.append((int(kx), start, prev))
                start = prev = v
        groups.append((int(kx), start, prev))
    return groups


def _span(groups):
    if not groups:
        return 1
    kxs = [t[0] for t in groups]
    return max(kxs) - min(kxs) + 1


def shear_ker(ker, sg):
    """ker'[ky, q] with q = kx + sg*(32 - ky) (shear; line angles in
    [0,90) have kx non-decreasing in ky, so this shrinks diagonal lines)."""
    K = ker.shape[0]
    wide = np.zeros((K, K + 32 * sg + 1), ker.dtype)
    for ky in range(K):
        s = sg * (32 - ky)
        wide[ky, s:s + K] = ker[ky]
    return wide


SIGMAS = (0, 1, 2)


def basis_plans(ker):
    """All (cost, groups, transposed, sigma) candidates, sorted by cost."""
    plans = []
    for tr in (False, True):
        km = ker.T if tr else ker
        for sg in SIGMAS:
            g = _col_groups(shear_ker(km, sg) if sg else km)
            cost = _span(g) * (WC + WEDGE * sg)
            plans.append((cost, g, tr, sg))
    plans.sort(key=lambda p: p[0])
    return plans


def sample_plan(tbl_ch0, amt_b, ang_b):
    """-> (scale, groups, transposed, sigma). groups are column-groups of
    the transformed kernel (column index q; true kx = q - 32*sigma +
    sigma*ky); basis minimizes span * streamed width."""
    rad = np.float32(ang_b * math.pi / 180.0)
    ker = rotate_nearest_np(tbl_ch0[amt_b], rad)
    ys, xs = np.nonzero(ker)
    if len(ys) == 0:
        return np.float32(0.0), [], False, 0
    scale = float(ker[ys[0], xs[0]])
    _, g, tr, sg = basis_plans(ker)[0]
    return np.float32(scale), g, tr, sg


def band_matrices(klo, khi):
    """w0 [128,112]: img rows 0..127 x out rows 0..111 (band r-y in
    [klo-15, khi-15]); w1: img rows 96..223 x out rows 112..223 (tile row
    r = img row 96+r, band r-y in [klo+1, khi+1]). Band clipping at the
    partition edges implements the vertical SAME padding."""
    r = np.arange(128)[:, None]
    y = np.arange(112)[None, :]
    d = r - y
    w0 = ((d >= klo - PAD_LO) & (d <= khi - PAD_LO)).astype(np.float32)
    w1 = ((d >= klo + 1) & (d <= khi + 1)).astype(np.float32)
    return w0, w1


def prepare_host(x, kernels_table, amt, angles, n_cores=8):
    B = x.shape[0]
    assert B % n_cores == 0
    slots = B // n_cores
    tbl_ch0 = np.ascontiguousarray(kernels_table[:, :, :, 0])

    scales = np.zeros(B, np.float32)
    groups = []
    transposed = np.zeros(B, bool)
    sigmas = np.zeros(B, np.int64)
    spans = np.zeros(B, np.int64)
    for b in range(B):
        s, g, tr, sg = sample_plan(tbl_ch0, int(amt[b]), int(angles[b]))
        scales[b] = s
        groups.append(g)
        transposed[b] = tr
        sigmas[b] = sg
        spans[b] = _span(g)

    # sigma must be uniform within a slot (SPMD). Round each sheared class
    # to a multiple of n_cores by demoting the samples with the smallest
    # cost penalty to a lower-sigma basis, then balance on span per class.
    for sg in (2, 1):
        idx = [b for b in range(B) if sigmas[b] == sg]
        rem = len(idx) % n_cores
        if not rem:
            continue
        pen = []
        for b in idx:
            ker = rotate_nearest_np(tbl_ch0[int(amt[b])],
                                    np.float32(int(angles[b]) * math.pi / 180.0))
            plans = [p for p in basis_plans(ker) if p[3] < sg]
            cur = spans[b] * (WC + WEDGE * sg)
            pen.append((plans[0][0] - cur, b, plans[0]))
        pen.sort(key=lambda t: t[0])
        for _, b, (cost, g, tr, s2) in pen[:rem]:
            groups[b], transposed[b], sigmas[b] = g, tr, s2
            spans[b] = _span(g)

    asg_rows = []
    row_sigma = []
    for sg in SIGMAS:
        idx = np.where(sigmas == sg)[0]
        if len(idx) == 0:
            continue
        assert len(idx) % n_cores == 0, (sg, len(idx))
        order = idx[np.argsort(-spans[idx], kind="stable")]
        rows = order.reshape(len(idx) // n_cores, n_cores)
        for r in rows:
            asg_rows.append(r)
            row_sigma.append(sg)
    asg = np.stack(asg_rows)
    row_sigma = np.array(row_sigma)
    assert asg.shape == (slots, n_cores)

    # schedule: lightest slot first (its input DMA completes fastest, so
    # the PE starts early), a light slot last (small tail), heavy middle.
    slot_cost = np.array([max(1, spans[asg[j]].max()) * (WC + WEDGE * row_sigma[j])
                          for j in range(slots)])
    order = np.argsort(-slot_cost, kind="stable")   # heavy .. light
    sched = np.concatenate([[order[-2]], order[:-2], [order[-1]]])
    asg = asg[sched]
    row_sigma = row_sigma[sched]

    gmax = np.array([max(1, spans[asg[j]].max()) for j in range(slots)])
    wout = WC + WEDGE * row_sigma                    # result width per row block
    wprime = 3 * gmax + wout                         # moving window width
    wprime = ((wprime + 7) // 8) * 8
    blobw = 4 * wprime + 224 * gmax                  # 4 image planes + wt table
    col_base = np.concatenate([[0], np.cumsum(blobw)])[:-1]
    totbw = int(blobw.sum())
    out_base = np.concatenate([[0], np.cumsum(2 * 112 * wout)])[:-1]
    totout = int((2 * 112 * wout).sum())

    # fp8 hi/lo planes of the full batch
    x8hi = x.astype(FP8)
    xlo = x - x8hi.astype(np.float32)
    x8lo = xlo.astype(FP8)

    in_maps = []
    mapping = np.zeros((n_cores, slots), np.int64)
    for c in range(n_cores):
        blob = np.zeros((128, totbw), FP8)
        scl = np.zeros((128, slots), np.float32)
        for j in range(slots):
            b = int(asg[j, c])
            G = int(gmax[j])
            sg = int(row_sigma[j])
            Wp = int(wprime[j])
            base = int(col_base[j])
            mapping[c, j] = b
            scl[:, j] = scales[b]

            if transposed[b]:
                hi = np.ascontiguousarray(x8hi[b].transpose(1, 0, 2)).reshape(H, WC)
                lo = np.ascontiguousarray(x8lo[b].transpose(1, 0, 2)).reshape(H, WC)
            else:
                hi = x8hi[b].reshape(H, WC)
                lo = x8lo[b].reshape(H, WC)
            phi = np.zeros((H, PIMG_W), FP8)
            plo = np.zeros((H, PIMG_W), FP8)
            phi[:, PIMG_PAD:PIMG_PAD + WC] = hi
            plo[:, PIMG_PAD:PIMG_PAD + WC] = lo

            # group code cols q: true kx = q - 32*sg + sg*ky
            bk = min(t[0] for t in groups[b]) if groups[b] else 0
            # window row p of block hb covers pimg cols
            #   V0 + 3*sg*p + [0, Wp); out tile col u (psum col) holds
            #   out[R+r, w + 3*sg*r + u] with w = -WEDGE*sg.
            # matching: rhs col u' = u + 3*(q - bk) reads tap (q, ky) when
            #   V0 = PIMG_PAD - 45 + w + 3*(bk - 32*sg) + 3*sg*(S + 15 - R)
            for hb, (R, S) in enumerate(((0, 0), (112, 96))):
                V0 = (PIMG_PAD - 45 - WEDGE * sg + 3 * (bk - 32 * sg)
                      + 3 * sg * (S + 15 - R))
                assert 0 <= V0 and V0 + 3 * sg * 127 + Wp <= PIMG_W, \
                    (V0, sg, bk, Wp)
                rows = np.arange(128)
                cols = V0 + 3 * sg * rows
                src_rows = S + rows
                for pl, pimg in enumerate((phi, plo)):
                    dst = base + (2 * hb + pl) * Wp
                    win = np.zeros((128, Wp), FP8)
                    for p in range(128):
                        sr = src_rows[p]
                        if 0 <= sr < H:
                            win[p] = pimg[sr, cols[p]:cols[p] + Wp]
                    blob[:, dst:dst + Wp] = win

            # weight table: [G, 2(hb), 112] fp8, code col q = bk + g
            wtb = base + 4 * Wp
            wcols = np.zeros((128, G, 2, 112), np.float32)
            for q, klo, khi in groups[b]:
                i = q - bk
                assert 0 <= i < G, (b, q, bk, G)
                w0, w1 = band_matrices(klo, khi)
                wcols[:, i, 0, :] += w0
                wcols[:, i, 1, :] += w1
            blob[:, wtb:wtb + 224 * G] = np.ascontiguousarray(wcols).reshape(128, 224 * G).astype(FP8)
        in_maps.append({"ximg": blob, "scl": scl})

    meta = {
        "slots": slots,
        "gmax": [int(v) for v in gmax],
        "sigma": [int(v) for v in row_sigma],
        "wout": [int(v) for v in wout],
        "wprime": [int(v) for v in wprime],
        "blobw": [int(v) for v in blobw],
        "col_base": [int(v) for v in col_base],
        "out_base": [int(v) for v in out_base],
        "totbw": totbw,
        "totout": totout,
        "mapping": mapping,
        "transposed": transposed,
    }
    return meta, in_maps


def _chunks(wout):
    """Split a result width into <=512-col PSUM chunks."""
    n = -(-wout // 512)
    w = -(-wout // n)
    out = []
    off = 0
    while off < wout:
        c = min(w, wout - off)
        out.append((off, c))
        off += c
    return out


# ---------------------------------------------------------------- device IR
def build_program(meta):
    import concourse.bacc as bacc
    import concourse.mybir as mybir
    from concourse.tile import TileContext
    from bass_rust import VecI64Pair

    fp8 = mybir.dt.float8e4
    slots = meta["slots"]

    nc = bacc.Bacc("TRN2")
    ximg = nc.dram_tensor("ximg", [128, meta["totbw"]], fp8, kind="ExternalInput")
    scl = nc.dram_tensor("scl", [128, slots], mybir.dt.float32,
                         kind="ExternalInput")
    out = nc.dram_tensor("out", [1, meta["totout"]], mybir.dt.float16,
                         kind="ExternalOutput")

    def strided(tile, dims, offset):
        ap = tile[:, 0:1].copy()
        ap.ap = VecI64Pair(dims)
        ap.offset = offset
        return ap

    with TileContext(nc) as tc:
        with tc.tile_pool(name="const", bufs=1) as cpool, \
             tc.tile_pool(name="img", bufs=10) as ipool, \
             tc.tile_pool(name="res", bufs=10) as rpool, \
             tc.tile_pool(name="ps0", bufs=2, space="PSUM") as pw0, \
             tc.tile_pool(name="ps1", bufs=2, space="PSUM") as pw1, \
             tc.tile_pool(name="ps2", bufs=2, space="PSUM") as pw2:
            st = cpool.tile([128, slots], mybir.dt.float32)
            nc.scalar.dma_start(out=st, in_=scl[:, :])

            wpools = [pw0, pw1, pw2]
            for j in range(slots):
                G = meta["gmax"][j]
                WO = meta["wout"][j]
                Wp = meta["wprime"][j]
                BW = meta["blobw"][j]
                base = meta["col_base"][j]
                obase = meta["out_base"][j]
                ch = _chunks(WO)
                blob = ipool.tile([128, BW], fp8, tag="blob", name="blob")
                wtb = 4 * Wp
                nc.sync.dma_start(out=blob, in_=ximg[:, base:base + BW])

                rt = rpool.tile([112, 2 * WO], mybir.dt.float16, tag="rt",
                                name="rt")
                sc = st[0:112, j:j + 1]
                for hb in (0, 1):
                    psums = [wpools[wh].tile([112, ch[wh][1]], mybir.dt.float32,
                                             tag=f"ps{wh}", name=f"ps{wh}")
                             for wh in range(len(ch))]
                    for g in range(G):
                        # same band matrix for both fp8 planes (hi, lo)
                        lhs = strided(blob, [[BW, 128], [0, 2], [1, 112]],
                                      wtb + 224 * g + 112 * hb)
                        for wh in range(len(ch)):
                            # planes (hi, lo) of window hb at column shift 3g
                            rhs = strided(
                                blob, [[BW, 128], [Wp, 2], [1, ch[wh][1]]],
                                2 * hb * Wp + 3 * g + ch[wh][0])
                            nc.tensor.matmul(
                                psums[wh], lhsT=lhs, rhs=rhs,
                                start=(g == 0), stop=(g == G - 1),
                                perf_mode=mybir.MatmulPerfMode.DoubleRow)
                    for wh in range(len(ch)):
                        dstc = rt[:, hb * WO + ch[wh][0]:
                                  hb * WO + ch[wh][0] + ch[wh][1]]
                        if wh == 0:
                            nc.scalar.activation(
                                out=dstc, in_=psums[wh],
                                func=mybir.ActivationFunctionType.Copy,
                                scale=sc)
                        else:
                            nc.vector.tensor_scalar_mul(out=dstc,
                                                        in0=psums[wh],
                                                        scalar1=sc)
                src = strided(rt, [[2 * WO, 112], [WO, 2], [1, WO]], 0)
                dst = out[0, 0:1].copy()
                dst.ap = VecI64Pair([[WO, 112], [112 * WO, 2], [1, WO]])
                dst.offset = obase
                nc.gpsimd.dma_start(out=dst, in_=src)
    return nc


def run_cores(meta, in_maps, trace=False):
    from concourse.bass_utils import run_bass_kernel_spmd

    nc = build_program(meta)
    nc.compile()
    dedupe_ldweights(nc)
    res = run_bass_kernel_spmd(nc, in_maps, core_ids=list(range(len(in_maps))),
                               trace=trace)
    return res


def unshard(meta, results):
    B = meta["mapping"].size
    out = np.zeros((B, H, W, C), np.float32)
    for c, r in enumerate(results):
        o = np.asarray(r["out"], np.float32).reshape(-1)
        for j in range(meta["slots"]):
            b = meta["mapping"][c, j]
            WO = meta["wout"][j]
            sg = meta["sigma"][j]
            t = o[meta["out_base"][j]:meta["out_base"][j] + 2 * 112 * WO]
            t = t.reshape(2, 112, WO)
            img = np.zeros((H, WC), np.float32)
            if sg == 0:
                img[0:112] = t[0, :, 0:WC]
                img[112:224] = t[1, :, 0:WC]
            else:
                for r_ in range(112):
                    u = 3 * sg * (111 - r_)
                    img[r_] = t[0, r_, u:u + WC]
                    img[112 + r_] = t[1, r_, u:u + WC]
            img = img.reshape(H, W, C)
            if meta["transposed"][b]:
                img = img.transpose(1, 0, 2)
            out[b] = img
    return out


def kernel(x, kernels_table, amt, angles):
    x = np.asarray(x, np.float32)
    kernels_table = np.asarray(kernels_table, np.float32)
    amt = np.asarray(amt)
    angles = np.asarray(angles)
    meta, in_maps = prepare_host(x, kernels_table, amt, angles)
    res = run_cores(meta, in_maps)
    return unshard(meta, res.results)



# revision 2
# speedup vs baseline: 1.6098x; 1.6098x over previous
"""Trainium2 Bass kernel for nn_BlurLayer (B=128, 224x224x3, per-sample
rotated-line motion blur, SAME depthwise conv).

Self-contained: kernel(**inputs) -> np.ndarray. Shards the batch over 8
NeuronCores (pure data parallel: 16 samples per core), compiles + runs one
SPMD Bass program via concourse.bass_utils.run_bass_kernel_spmd, gathers
the full output.

Method: the rotated blur kernel's nonzero taps all equal 1/size and form a
digitized line. The conv is computed as G matmuls per 112-row output block:
out[r, u] += W_g[p, r] * window[p, u + 3g], where the window rows carry a
per-row horizontal shift sigma(m) and the psum columns carry a per-row
output shift omega(Y) (both baked host-side into the blob / unshard, so
all device access patterns are static). sigma and omega are chosen per
sample by a Bellman-Ford difference-constraint solver to minimize G (the
number of distinct shifted column groups needed to cover every tap) -- for
any line angle this lands at G in {1,2,3} instead of the raw column span.
The masks W_g are arbitrary per-sample 0/1 matrices (clipped at the block
edges, which implements the vertical SAME padding). The image is split
into two fp8e4m3 planes (hi = fp8(x), lo = fp8(x - hi)) so each matmul
runs in DoubleRow perf mode; 1/size scaling happens in the PSUM->SBUF
copies, split between the Scalar and Vector engines. Each slot's windows +
mask table arrive in one DMA; a post-compile pass drops back-to-back
reloads of identical PE weights; outputs DMA out per half-block to shrink
the tail.
"""

import math

import numpy as np
import ml_dtypes

MAXK = 32
H = W = 224
C = 3
WC = W * C                  # 672
PAD_LO = (MAXK - 1) // 2    # 15
PIMG_PAD = 800              # left zero margin (elems) of padded image row
PIMG_W = PIMG_PAD + WC + 800

FP8 = ml_dtypes.float8_e4m3


def dedupe_ldweights(nc):
    """Replace an InstLdweights whose weights AP is identical to the
    immediately-preceding one (per block) with an InstNoOp carrying its
    sync_info: the PE array still holds those weights, so the reload is
    pure Tensor-queue overhead (~158ns each)."""
    import concourse.mybir as mybir
    n = 0
    for fn in nc.m.functions:
        for blk in fn.blocks:
            prev_key = None
            new_insts = []
            for inst in blk.instructions:
                if isinstance(inst, mybir.InstLdweights):
                    key = (repr(inst.ins), repr(getattr(inst, "perf_mode", None)),
                           repr(getattr(inst, "is_transpose", None)))
                    if key == prev_key:
                        n += 1
                        inst = mybir.InstNoOp(
                            name=f"{inst.name}-ldwdedup",
                            engine=inst.engine,
                            ins=[], outs=[],
                            sync_info=inst.sync_info,
                        )
                    else:
                        prev_key = key
                new_insts.append(inst)
            blk.instructions = new_insts
    return n


# ---------------------------------------------------------------- host math
def rotate_nearest_np(img, rad):
    K = img.shape[0]
    cos, sin = np.cos(rad), np.sin(rad)
    coords = np.arange(K, dtype=np.float32)
    yy, xx = np.meshgrid(coords, coords, indexing="ij")
    e = np.float32(K - 1)
    x_off = (e - (cos * e - sin * e)) * 0.5
    y_off = (e - (sin * e + cos * e)) * 0.5
    sx = cos * xx - sin * yy + x_off
    sy = sin * xx + cos * yy + y_off
    ix = np.round(sx).astype(np.int32)
    iy = np.round(sy).astype(np.int32)
    valid = (ix >= 0) & (ix < K) & (iy >= 0) & (iy < K)
    g = img[np.clip(iy, 0, K - 1), np.clip(ix, 0, K - 1)]
    return np.where(valid, g, np.float32(0.0))


def tap_pairs(ker):
    """Valid (m, Y, kx) triples: img row m = Y + ky - 15 for out row Y and
    tap (ky, kx), both m and Y in [0, 224)."""
    ys, xs = np.nonzero(ker)
    Y = np.arange(H)
    M, YY, KX = [], [], []
    for ky, kx in zip(ys, xs):
        m = Y + ky - 15
        ok = (m >= 0) & (m < H)
        M.append(m[ok])
        YY.append(Y[ok])
        KX.append(np.full(int(ok.sum()), kx))
    return np.concatenate(M), np.concatenate(YY), np.concatenate(KX)


def solve_shear(ker, max_rounds=400):
    """Choose integer shift profiles sigma (img rows) / nu (out rows)
    minimizing G = #values of g = kx - sigma[m] + nu[Y] over all taps.
    Difference-constraint feasibility via vectorized Bellman-Ford.
    Returns (G, sigma[224], nu[224], c) with g - c in [0, G)."""
    M, Y, KX = tap_pairs(ker)
    key = M * H + Y
    order = np.argsort(key)
    key_s, kx_s = key[order], KX[order]
    uk, idx = np.unique(key_s, return_index=True)
    lo = np.minimum.reduceat(kx_s, idx)
    hi = np.maximum.reduceat(kx_s, idx)
    um, uy = uk // H, uk % H
    G0 = int((hi - lo).max()) + 1
    for G in range(G0, 40):
        src = np.concatenate([224 + uy, um])
        dst = np.concatenate([um, 224 + uy])
        w = np.concatenate([lo, (G - 1) - hi]).astype(np.int64)
        dist = np.zeros(448, np.int64)
        ok = False
        for _ in range(max_rounds):
            nd = dist.copy()
            np.minimum.at(nd, dst, dist[src] + w)
            if np.array_equal(nd, dist):
                ok = True
                break
            dist = nd
        if ok:
            sigma, nu = dist[:224], dist[224:]
            g = KX - sigma[M] + nu[Y]
            c = int(g.min())
            assert int(g.max()) - c + 1 <= G
            return int(g.max()) - c + 1, sigma, nu, c
    raise RuntimeError("no feasible G")


def sample_plan(tbl_ch0, amt_b, ang_b):
    """-> dict with scale, taps, G, sigma, nu, c, wout, transposed."""
    rad = np.float32(ang_b * math.pi / 180.0)
    ker0 = rotate_nearest_np(tbl_ch0[amt_b], rad)
    ys, xs = np.nonzero(ker0)
    scale = float(ker0[ys[0], xs[0]])
    best = None
    for tr in (False, True):
        km = ker0.T if tr else ker0
        G, sigma, nu, c = solve_shear(km)
        w0 = int(nu[0:112].max() - nu[0:112].min())
        w1 = int(nu[112:224].max() - nu[112:224].min())
        wout = WC + 3 * max(w0, w1)
        cost = 2 * G * wout
        if best is None or cost < best["cost"]:
            best = dict(cost=cost, G=G, sigma=sigma, nu=nu, c=c, wout=wout,
                        transposed=tr, ker=km, scale=np.float32(scale))
    return best


def prepare_host(x, kernels_table, amt, angles, n_cores=8):
    B = x.shape[0]
    assert B % n_cores == 0
    slots = B // n_cores
    tbl_ch0 = np.ascontiguousarray(kernels_table[:, :, :, 0])

    plans = [sample_plan(tbl_ch0, int(amt[b]), int(angles[b]))
             for b in range(B)]
    Gs = np.array([p["G"] for p in plans])
    wos = np.array([p["wout"] for p in plans])

    # slot packing: G uniform-ish per slot (sort by G then wout, rows of 8)
    order = np.lexsort((-wos, -Gs))
    asg = order.reshape(slots, n_cores)

    slotG = np.array([Gs[asg[j]].max() for j in range(slots)])
    slotW = np.array([wos[asg[j]].max() for j in range(slots)])

    # schedule: a light slot first (its input DMA completes fastest, so
    # the PE starts early), the lightest last (small tail), heavy middle.
    slot_cost = 2 * slotG * slotW
    o = np.argsort(-slot_cost, kind="stable")   # heavy .. light
    sched = np.concatenate([[o[-2]], o[:-2], [o[-1]]])
    asg = asg[sched]
    slotG = slotG[sched]
    slotW = slotW[sched]

    gmax = slotG
    wout = slotW
    wprime = ((wout + 3 * gmax + 7) // 8) * 8
    blobw = 4 * wprime + 224 * gmax            # 4 image planes + mask table
    col_base = np.concatenate([[0], np.cumsum(blobw)])[:-1]
    totbw = int(blobw.sum())
    out_base = np.concatenate([[0], np.cumsum(2 * 112 * wout)])[:-1]
    totout = int((2 * 112 * wout).sum())

    # fp8 hi/lo planes of the full batch
    x8hi = x.astype(FP8)
    xlo = x - x8hi.astype(np.float32)
    x8lo = xlo.astype(FP8)

    in_maps = []
    mapping = np.zeros((n_cores, slots), np.int64)
    omega_all = np.zeros((n_cores, slots, 2, 112), np.int64)
    for cidx in range(n_cores):
        blob = np.zeros((128, totbw), FP8)
        scl = np.zeros((128, slots), np.float32)
        for j in range(slots):
            b = int(asg[j, cidx])
            p = plans[b]
            G = int(gmax[j])
            Wp = int(wprime[j])
            base = int(col_base[j])
            mapping[cidx, j] = b
            scl[:, j] = p["scale"]
            sigma, nu, c = p["sigma"], p["nu"], p["c"]

            if p["transposed"]:
                hi = np.ascontiguousarray(x8hi[b].transpose(1, 0, 2)).reshape(H, WC)
                lo = np.ascontiguousarray(x8lo[b].transpose(1, 0, 2)).reshape(H, WC)
            else:
                hi = x8hi[b].reshape(H, WC)
                lo = x8lo[b].reshape(H, WC)
            phi = np.zeros((H, PIMG_W), FP8)
            plo = np.zeros((H, PIMG_W), FP8)
            phi[:, PIMG_PAD:PIMG_PAD + WC] = hi
            plo[:, PIMG_PAD:PIMG_PAD + WC] = lo

            for hb, (R, S) in enumerate(((0, 0), (112, 96))):
                numax = int(nu[R:R + 112].max())
                omega_all[cidx, j, hb] = numax - nu[R:R + 112]
                V0 = PIMG_PAD + 3 * (c - PAD_LO - numax)
                rows = np.arange(128)
                cols = V0 + 3 * sigma[S + rows]
                assert cols.min() >= 0 and cols.max() + Wp <= PIMG_W, \
                    (b, hb, cols.min(), cols.max(), Wp)
                for pl, pimg in enumerate((phi, plo)):
                    dst = base + (2 * hb + pl) * Wp
                    win = np.zeros((128, Wp), FP8)
                    for pp in range(128):
                        win[pp] = pimg[S + pp, cols[pp]:cols[pp] + Wp]
                    blob[:, dst:dst + Wp] = win

            # mask table: [128, G, 2(hb), 112] fp8
            wtb = base + 4 * Wp
            wcols = np.zeros((128, G, 2, 112), np.float32)
            ys, xs = np.nonzero(p["ker"])
            r = np.arange(112)
            for ky, kx in zip(ys, xs):
                for hb, (R, S) in enumerate(((0, 0), (112, 96))):
                    m = R + r + ky - PAD_LO
                    pr = m - S
                    ok = (pr >= 0) & (pr < 128) & (m >= 0) & (m < H)
                    if not ok.any():
                        continue
                    g = (kx - sigma[m[ok]] + nu[R + r[ok]]) - c
                    assert g.min() >= 0 and g.max() < G, (b, hb, g.min(), g.max(), G)
                    wcols[pr[ok], g, hb, r[ok]] = 1.0
            blob[:, wtb:wtb + 224 * G] = np.ascontiguousarray(wcols).reshape(
                128, 224 * G).astype(FP8)
        in_maps.append({"ximg": blob, "scl": scl})

    meta = {
        "slots": slots,
        "gmax": [int(v) for v in gmax],
        "wout": [int(v) for v in wout],
        "wprime": [int(v) for v in wprime],
        "blobw": [int(v) for v in blobw],
        "col_base": [int(v) for v in col_base],
        "out_base": [int(v) for v in out_base],
        "totbw": totbw,
        "totout": totout,
        "mapping": mapping,
        "omega": omega_all,
        "transposed": np.array([p["transposed"] for p in plans]),
    }
    return meta, in_maps


def _chunks(wout):
    """Split a result width into <=512-col PSUM chunks."""
    n = -(-wout // 512)
    w = -(-wout // n)
    out = []
    off = 0
    while off < wout:
        cc = min(w, wout - off)
        out.append((off, cc))
        off += cc
    return out


# ---------------------------------------------------------------- device IR
def build_program(meta):
    import concourse.bacc as bacc
    import concourse.mybir as mybir
    from concourse.tile import TileContext
    from bass_rust import VecI64Pair

    fp8 = mybir.dt.float8e4
    slots = meta["slots"]

    nc = bacc.Bacc("TRN2")
    ximg = nc.dram_tensor("ximg", [128, meta["totbw"]], fp8, kind="ExternalInput")
    scl = nc.dram_tensor("scl", [128, slots], mybir.dt.float32,
                         kind="ExternalInput")
    out = nc.dram_tensor("out", [1, meta["totout"]], mybir.dt.float16,
                         kind="ExternalOutput")

    def strided(tile, dims, offset):
        ap = tile[:, 0:1].copy()
        ap.ap = VecI64Pair(dims)
        ap.offset = offset
        return ap

    with TileContext(nc) as tc:
        with tc.tile_pool(name="const", bufs=1) as cpool, \
             tc.tile_pool(name="img", bufs=10) as ipool, \
             tc.tile_pool(name="res", bufs=10) as rpool, \
             tc.tile_pool(name="ps0", bufs=2, space="PSUM") as pw0, \
             tc.tile_pool(name="ps1", bufs=2, space="PSUM") as pw1, \
             tc.tile_pool(name="ps2", bufs=2, space="PSUM") as pw2:

            st = cpool.tile([128, slots], mybir.dt.float32)
            nc.scalar.dma_start(out=st, in_=scl[:, :])

            wpools = [pw0, pw1, pw2]
            for j in range(slots):
                G = meta["gmax"][j]
                WO = meta["wout"][j]
                Wp = meta["wprime"][j]
                BW = meta["blobw"][j]
                base = meta["col_base"][j]
                obase = meta["out_base"][j]
                ch = _chunks(WO)
                blob = ipool.tile([128, BW], fp8, tag="blob", name="blob")
                wtb = 4 * Wp
                nc.sync.dma_start(out=blob, in_=ximg[:, base:base + BW])

                sc = st[0:112, j:j + 1]
                for hb in (0, 1):
                    rt = rpool.tile([112, WO], mybir.dt.float16, tag=f"rt{hb}",
                                    name=f"rt{hb}")
                    psums = [wpools[wh].tile([112, ch[wh][1]], mybir.dt.float32,
                                             tag=f"ps{wh}", name=f"ps{wh}")
                             for wh in range(len(ch))]
                    for g in range(G):
                        # same mask for both fp8 planes (hi, lo)
                        lhs = strided(blob, [[BW, 128], [0, 2], [1, 112]],
                                      wtb + 224 * g + 112 * hb)
                        for wh in range(len(ch)):
                            # planes (hi, lo) of window hb at column shift 3g
                            rhs = strided(
                                blob, [[BW, 128], [Wp, 2], [1, ch[wh][1]]],
                                2 * hb * Wp + 3 * g + ch[wh][0])
                            nc.tensor.matmul(
                                psums[wh], lhsT=lhs, rhs=rhs,
                                start=(g == 0), stop=(g == G - 1),
                                perf_mode=mybir.MatmulPerfMode.DoubleRow)
                    for wh in range(len(ch)):
                        dstc = rt[:, ch[wh][0]:ch[wh][0] + ch[wh][1]]
                        if wh == 0:
                            nc.scalar.activation(
                                out=dstc, in_=psums[wh],
                                func=mybir.ActivationFunctionType.Copy,
                                scale=sc)
                        else:
                            nc.vector.tensor_scalar_mul(out=dstc,
                                                        in0=psums[wh],
                                                        scalar1=sc)
                    src = strided(rt, [[WO, 112], [1, WO]], 0)
                    dst = out[0, 0:1].copy()
                    dst.ap = VecI64Pair([[WO, 112], [1, WO]])
                    dst.offset = obase + hb * 112 * WO
                    nc.gpsimd.dma_start(out=dst, in_=src)
    return nc


def run_cores(meta, in_maps, trace=False):
    from concourse.bass_utils import run_bass_kernel_spmd

    nc = build_program(meta)
    nc.compile()
    dedupe_ldweights(nc)
    res = run_bass_kernel_spmd(nc, in_maps, core_ids=list(range(len(in_maps))),
                               trace=trace)
    return res


def unshard(meta, results):
    B = meta["mapping"].size
    out = np.zeros((B, H, W, C), np.float32)
    for cidx, r in enumerate(results):
        o = np.asarray(r["out"], np.float32).reshape(-1)
        for j in range(meta["slots"]):
            b = meta["mapping"][cidx, j]
            WO = meta["wout"][j]
            t = o[meta["out_base"][j]:meta["out_base"][j] + 2 * 112 * WO]
            t = t.reshape(2, 112, WO)
            img = np.zeros((H, WC), np.float32)
            om = meta["omega"][cidx, j]
            for hb in (0, 1):
                for r_ in range(112):
                    u = 3 * int(om[hb, r_])
                    img[112 * hb + r_] = t[hb, r_, u:u + WC]
            img = img.reshape(H, W, C)
            if meta["transposed"][b]:
                img = img.transpose(1, 0, 2)
            out[b] = img
    return out


def kernel(x, kernels_table, amt, angles):
    x = np.asarray(x, np.float32)
    kernels_table = np.asarray(kernels_table, np.float32)
    amt = np.asarray(amt)
    angles = np.asarray(angles)
    meta, in_maps = prepare_host(x, kernels_table, amt, angles)
    res = run_cores(meta, in_maps)
    return unshard(meta, res.results)
